# revision 1
# baseline (speedup 1.0000x reference)
"""Trainium2 Bass kernel for nn_ConvolutionalAttention_3015067042131.

Math (reference.py):
  x [16,128,64,64] f32; x1 = x[:, :64], x2 = x[:, 64:]
  pooled = mean(x1, HW); h = gelu(pooled @ w1.T + b1); dyn = (h @ w2.T + b2) -> [B,64,9]
  x1_dyn = per-(batch,channel) 3x3 depthwise conv of x1 with dyn
  x1_lk  = conv2d(x1, lk_filter[64,64,13,13], SAME)
  out = concat([x1_lk + x1_dyn, x2], ch)

Strategy:
  * The tiny MLP (dyn) is computed on host in float64 (0.0007% of FLOPs).
  * The dynamic depthwise 3x3 is folded into the 13x13 conv weights as
    per-batch diagonal additions on the central 3x3 taps (3x3 tap (u,v)
    == 13x13 tap (u+5, v+5)); so the device runs ONE dense 13x13 conv
    with per-batch weights on 6 of 91 weight tiles.
  * Conv as shift-and-matmul: for each kernel tap, out[o, pix] +=
    W_tap[c, o].T @ xpad[c, pix+off]. Taps are paired along K: SBUF
    partitions 0-63 hold the zero-padded image (76x76), partitions
    64-127 hold it shifted one column left, so taps (i,j) and (i,j+1)
    fuse into one K=128 matmul. 91 matmuls cover all 169 taps.
  * Output pixels processed in 8 chunks of 512 (8 rows). Chunk pairs run
    CONCURRENTLY in the two PE column halves via tile_position (0,0) /
    (0,64) writing PSUM partitions 0-63 / 64-127 (measured 2x).
  * fp16 operands (measured: HW fp16 matmul exact on rounded inputs,
    fp32 PSUM accumulate; end-to-end rel err ~3e-4). f32r is broken in
    this stack (device-crashing) and fp32 runs at 1/4 rate.
  * Sharding: data-parallel over batch, 2 batches per core on 8 cores.
    x2 passthrough is host-side (no device work).
"""
import math

import numpy as np

B, C, H, W = 16, 128, 64, 64
PDIM, SK, LK = 64, 3, 13
PAD = LK // 2  # 6
HP, WP = H + 2 * PAD, W + 2 * PAD  # 76, 76
NCORES = 8
BPC = B // NCORES  # batches per core
NP = 91            # weight tiles (78 tap pairs + 13 singles)
NCHUNK = 8         # 512-pixel chunks per image
CHUNK = H * W // NCHUNK  # 512

# tile t = i*7 + p: p in 0..5 -> pair ((i,2p),(i,2p+1)); p == 6 -> single (i,12)
_TAP_A = {}
for _i in range(LK):
    for _p in range(7):
        _TAP_A[_i * 7 + _p] = (_i, 2 * _p if _p < 6 else 12)

# central 3x3 taps (i,j in 5..7) live in pair tiles i*7+2 (B-half j=5) and
# i*7+3 (A-half j=6, B-half j=7); those 6 tiles are per-batch.
_MOD_TILES = [5 * 7 + 2, 6 * 7 + 2, 7 * 7 + 2, 5 * 7 + 3, 6 * 7 + 3, 7 * 7 + 3]
_MOD_SLOT = {t: s for s, t in enumerate(_MOD_TILES)}

_ERF = np.vectorize(math.erf, otypes=[np.float64])

_CACHED_NC = None


def _build_nc():
    import concourse.mybir as mybir
    import concourse.tile as tile
    from concourse import bacc

    f32 = mybir.dt.float32
    f16 = mybir.dt.float16

    nc = bacc.Bacc(None, target_bir_lowering=False)
    xs = nc.dram_tensor("xs", [BPC, PDIM, H, W], f16, kind="ExternalInput")
    wsh = nc.dram_tensor("wsh", [128, NP * 64], f16, kind="ExternalInput")
    wmod = nc.dram_tensor("wmod", [BPC, 128, 6 * 64], f16, kind="ExternalInput")
    y = nc.dram_tensor("y", [BPC, PDIM, H * W], f32, kind="ExternalOutput")

    with tile.TileContext(nc) as tc:
        with (
            tc.tile_pool(name="wpool", bufs=1) as wpool,
            tc.tile_pool(name="wmpool", bufs=2) as wmpool,
            tc.tile_pool(name="xpool", bufs=2) as xpool,
            tc.tile_pool(name="opool", bufs=3) as opool,
            tc.tile_pool(name="pspool", bufs=4, space="PSUM") as pspool,
        ):
            wsh_sb = wpool.tile([128, NP * 64], f16)
            nc.sync.dma_start(out=wsh_sb[:], in_=wsh[:])

            # PE warmup: ~10 junk matmuls on a zeroed scratch tile so the
            # HAM un-throttles (1.2 -> 2.4 GHz) while the input DMAs run.
            scratch = wpool.tile([128, CHUNK], f16)
            nc.vector.memset(scratch[:], 0.0)
            ps_warm = pspool.tile([128, CHUNK], f32, name="ps_warm", bufs=1)
            for wi in range(26):
                nc.tensor.matmul(
                    ps_warm[0:64, :],
                    lhsT=scratch[:, 0:64],
                    rhs=scratch[:, :],
                    start=(wi == 0),
                    stop=(wi == 25),
                    skip_group_check=True,
                )

            for b in range(BPC):
                wm = wmpool.tile([128, 6 * 64], f16)
                nc.sync.dma_start(out=wm[:], in_=wmod[b, :, :])
                # Contiguous DMA (8KB runs/partition, fast) of the image into
                # BOTH partition halves of a staging tile, on two queues; the
                # strided padded layout is then built on-chip by DVE (the
                # direct strided DMA measured ~10x slower).
                xst = xpool.tile([128, H, W], f16, name="xst")
                nc.sync.dma_start(out=xst[0:64, :, :], in_=xs[b, :, :, :])
                nc.sync.dma_start(out=xst[64:128, :, :], in_=xs[b, :, :, :])
                xp = xpool.tile([128, HP, WP], f16)
                # border-only memsets, disjoint from the copied interiors so
                # nothing serializes behind them
                nc.vector.memset(xp[:, 0:PAD, :], 0.0)              # top rows
                nc.vector.memset(xp[:, PAD + H :, :], 0.0)          # bottom rows
                nc.vector.memset(xp[0:64, PAD : PAD + H, 0:PAD], 0.0)
                nc.vector.memset(xp[0:64, PAD : PAD + H, PAD + W :], 0.0)
                nc.vector.memset(xp[64:128, PAD : PAD + H, 0 : PAD - 1], 0.0)
                nc.vector.memset(xp[64:128, PAD : PAD + H, PAD - 1 + W :], 0.0)
                # partitions 0-63: padded image; 64-127: shifted left 1 col
                nc.vector.tensor_copy(
                    xp[0:64, PAD : PAD + H, PAD : PAD + W], xst[0:64, :, :]
                )
                # scalar engine so both halves reshape concurrently
                nc.scalar.copy(
                    xp[64:128, PAD : PAD + H, PAD - 1 : PAD - 1 + W],
                    xst[64:128, :, :],
                )
                for cp in range(NCHUNK // 2):
                    ps = pspool.tile([128, CHUNK], f32)
                    for t in range(NP):
                        s = _MOD_SLOT.get(t)
                        w_ap = (
                            wm[:, s * 64 : (s + 1) * 64]
                            if s is not None
                            else wsh_sb[:, t * 64 : (t + 1) * 64]
                        )
                        i, j = _TAP_A[t]
                        for half in (0, 1):
                            r0 = i + 8 * (2 * cp + half)
                            nc.tensor.matmul(
                                ps[64 * half : 64 * (half + 1), :],
                                lhsT=w_ap,
                                rhs=xp[:, r0 : r0 + 8, j : j + 64],
                                start=(t == 0),
                                stop=(t == NP - 1),
                                tile_position=(0, 64 * half),
                                skip_group_check=True,
                            )
                    ot = opool.tile([128, CHUNK], f32)
                    nc.vector.tensor_copy(ot[:], ps[:])
                    nc.sync.dma_start(
                        out=y[b, :, (2 * cp) * CHUNK : (2 * cp + 1) * CHUNK],
                        in_=ot[0:64, :],
                    )
                    nc.sync.dma_start(
                        out=y[b, :, (2 * cp + 1) * CHUNK : (2 * cp + 2) * CHUNK],
                        in_=ot[64:128, :],
                    )
    nc.compile()
    return nc


def _get_nc():
    global _CACHED_NC
    if _CACHED_NC is None:
        _CACHED_NC = _build_nc()
    return _CACHED_NC


def _host_dyn(x, w1, b1, w2, b2):
    """dwc_proj MLP on host, float64: dyn [B, 64, 9]."""
    pooled = x[:, :PDIM].mean(axis=(2, 3), dtype=np.float64)      # [B, 64]
    z = pooled @ w1.T.astype(np.float64) + b1.astype(np.float64)  # [B, 32]
    h = 0.5 * z * (1.0 + _ERF(z / math.sqrt(2.0)))                # exact gelu
    dyn = h @ w2.T.astype(np.float64) + b2.astype(np.float64)     # [B, 576]
    return dyn.reshape(B, PDIM, SK * SK)


def _host_weights(lk_filter, dyn):
    """Build shared tap-pair weight tiles + per-batch modified central tiles.

    Weight tile t [128, 64]: rows 0-63 = lk[o, c, iA, jA].T (tap A), rows
    64-127 = tap B = (iA, jA+1), zeros for singles. lhsT layout [K=c, M=o].
    """
    lkT = lk_filter.transpose(1, 0, 2, 3).astype(np.float32)  # [c, o, i, j]
    Wt = np.zeros((NP, 128, 64), np.float32)
    for t, (i, jA) in _TAP_A.items():
        Wt[t, 0:64, :] = lkT[:, :, i, jA]
        if jA < 12:
            Wt[t, 64:128, :] = lkT[:, :, i, jA + 1]

    ar = np.arange(64)
    Wmod = np.zeros((B, 6, 128, 64), np.float32)
    for ii, i in enumerate((5, 6, 7)):
        t2, t3 = i * 7 + 2, i * 7 + 3
        u = i - 5
        for b in range(B):
            m2 = Wt[t2].copy()
            m3 = Wt[t3].copy()
            m2[64 + ar, ar] += dyn[b, :, u * 3 + 0].astype(np.float32)  # tap (i,5)
            m3[ar, ar] += dyn[b, :, u * 3 + 1].astype(np.float32)       # tap (i,6)
            m3[64 + ar, ar] += dyn[b, :, u * 3 + 2].astype(np.float32)  # tap (i,7)
            Wmod[b, ii] = m2
            Wmod[b, 3 + ii] = m3

    wsh_np = np.ascontiguousarray(
        Wt.transpose(1, 0, 2).reshape(128, NP * 64)
    ).astype(np.float16)
    wmod_np = np.ascontiguousarray(
        Wmod.transpose(0, 2, 1, 3).reshape(B, 128, 6 * 64)
    ).astype(np.float16)
    return wsh_np, wmod_np


def kernel(x, lk_filter, w1, b1, w2, b2):
    from concourse.bass_utils import run_bass_kernel_spmd

    x = np.asarray(x, dtype=np.float32)
    dyn = _host_dyn(x, np.asarray(w1), np.asarray(b1), np.asarray(w2), np.asarray(b2))
    wsh_np, wmod_np = _host_weights(np.asarray(lk_filter, dtype=np.float32), dyn)

    x1_f16 = x[:, :PDIM].astype(np.float16)  # [16, 64, 64, 64]

    nc = _get_nc()
    in_maps = []
    for k in range(NCORES):
        b0 = k * BPC
        in_maps.append(
            {
                "xs": np.ascontiguousarray(x1_f16[b0 : b0 + BPC]),
                "wsh": wsh_np,
                "wmod": np.ascontiguousarray(wmod_np[b0 : b0 + BPC]),
            }
        )
    res = run_bass_kernel_spmd(nc, in_maps, core_ids=list(range(NCORES)))

    out = np.empty((B, C, H, W), np.float32)
    for k in range(NCORES):
        b0 = k * BPC
        out[b0 : b0 + BPC, :PDIM] = res.results[k]["y"].reshape(BPC, PDIM, H, W)
    out[:, PDIM:] = x[:, PDIM:]
    return out



# revision 5
# speedup vs baseline: 1.0706x; 1.0706x over previous
"""Trainium2 Bass kernel for nn_ConvolutionalAttention_3015067042131.

Math (reference.py):
  x [16,128,64,64] f32; x1 = x[:, :64], x2 = x[:, 64:]
  pooled = mean(x1, HW); h = gelu(pooled @ w1.T + b1); dyn = (h @ w2.T + b2) -> [B,64,9]
  x1_dyn = per-(batch,channel) 3x3 depthwise conv of x1 with dyn
  x1_lk  = conv2d(x1, lk_filter[64,64,13,13], SAME)
  out = concat([x1_lk + x1_dyn, x2], ch)

Strategy:
  * The tiny MLP (dyn) is computed on host in float64 (0.0007% of FLOPs).
  * The dynamic depthwise 3x3 is folded into the 13x13 conv weights as
    per-batch diagonal additions on the central 3x3 taps (3x3 tap (u,v)
    == 13x13 tap (u+5, v+5)).
  * Conv as shift-and-matmul: for each kernel tap, out[o, pix] +=
    W_tap[c, o].T @ xpad[c, pix+off]. Taps are paired along K=128:
      - xp layout: partitions 0-63 hold the zero-padded 76x76 image,
        64-127 hold it shifted LEFT one column -> taps (i,2j),(i,2j+1)
        fuse into one matmul. 78 tiles cover columns 0-11.
      - xq layout: partitions 64-127 hold the image shifted UP one row
        -> taps (2i,12),(2i+1,12) fuse. 6 tiles cover column 12 rows
        0-11, plus 1 single tile for tap (12,12).
    85 tiles total for 169 taps (optimal: 84 pairs + 1 single).
  * Loop order: batch outer, tap-tile outer, chunk inner. Each batch's
    full output (4 chunk-pairs x [128,512] f32) stays RESIDENT in PSUM
    (8 banks = exactly 2 images), accumulating across all 85 taps; one
    drain at the end of each batch, pipelined per bank.
  * Chunk pairs run CONCURRENTLY in the two PE column halves via
    tile_position (0,0)/(0,64) writing PSUM partitions 0-63/64-127.
  * fp16 operands (HW fp16 matmul, fp32 PSUM accumulate; end-to-end rel
    err ~3e-4). f32r is broken in this stack; fp32 runs at 1/4 rate;
    fp8 measured 3.8e-2 rel err on this data -> over the 2e-2 gate.
  * Head: the framework preamble is ~7.2us; immediately after it, junk
    matmuls ramp the PE clock (1.2->2.4GHz takes ~3us of sustained PE
    activity) while image DMAs (sync queue) + weight DMAs (scalar
    queue) land and DVE builds the padded layouts. Border memsets run
    during the DMA wait; per-batch layouts build one batch ahead.
  * Sharding: data-parallel over batch, 2 batches per core on 8 cores.
    x2 passthrough is host-side (no device work).
"""
import math

import numpy as np

B, C, H, W = 16, 128, 64, 64
PDIM, SK, LK = 64, 3, 13
PAD = LK // 2  # 6
HP, WP = H + 2 * PAD, W + 2 * PAD  # 76, 76
NCORES = 8
BPC = B // NCORES  # batches per core
NT = 85            # weight tiles (84 tap pairs + 1 single)
NCHUNK = 8         # 512-pixel chunks per image
CHUNK = H * W // NCHUNK  # 512
NWARM = 14         # clock-ramp junk matmuls (N=256 each)

# tile t: t < 78 -> col-pair, A tap (t//6, 2*(t%6)), B = (i, j+1), layout xp
#         78 <= t < 84 -> row-pair, A tap (2*(t-78), 12), B = (i+1, 12), xq
#         t == 84 -> single tap (12, 12), B-half weights zero, layout xp


def _tile_tap(t):
    if t < 78:
        return t // 6, 2 * (t % 6)
    if t < 84:
        return 2 * (t - 78), 12
    return 12, 12


# central 3x3 taps (i,j in 5..7): j=5 -> tile i*6+2 B-half; j=6/7 -> tile
# i*6+3 A/B-half; those 6 tiles are per-batch.
_MOD_TILES = [5 * 6 + 2, 6 * 6 + 2, 7 * 6 + 2, 5 * 6 + 3, 6 * 6 + 3, 7 * 6 + 3]
_MOD_SLOT = {t: s for s, t in enumerate(_MOD_TILES)}

_ERF = np.vectorize(math.erf, otypes=[np.float64])

_CACHED_NC = None


def _build_nc():
    import concourse.mybir as mybir
    import concourse.tile as tile
    from concourse import bacc

    f32 = mybir.dt.float32
    f16 = mybir.dt.float16

    nc = bacc.Bacc(None, target_bir_lowering=False)
    xs = nc.dram_tensor("xs", [BPC, PDIM, H, W], f16, kind="ExternalInput")
    wsh = nc.dram_tensor("wsh", [128, NT * 64], f16, kind="ExternalInput")
    wmod = nc.dram_tensor("wmod", [BPC, 128, 6 * 64], f16, kind="ExternalInput")
    y = nc.dram_tensor("y", [BPC, PDIM, H * W], f32, kind="ExternalOutput")

    with tile.TileContext(nc) as tc:
        with (
            tc.tile_pool(name="wpool", bufs=1) as wpool,
            tc.tile_pool(name="wmpool", bufs=1) as wmpool,
            tc.tile_pool(name="xstpool", bufs=1) as xstpool,
            tc.tile_pool(name="xppool", bufs=1) as xppool,
            tc.tile_pool(name="xqpool", bufs=1) as xqpool,
            tc.tile_pool(name="opool", bufs=1) as opool,
            tc.tile_pool(name="pspool", bufs=1, space="PSUM") as pspool,
        ):
            # 8 PSUM banks = exactly the per-core output (2 images x 4
            # chunk-pair banks); resident for the whole batch accumulation
            ps_t = [
                [
                    pspool.tile([128, CHUNK], f32, name=f"ps{b}{cp}")
                    for cp in range(4)
                ]
                for b in range(BPC)
            ]
            # PE warmup: junk matmuls on a zeroed scratch tile so the HAM
            # un-throttles (1.2 -> 2.4 GHz) while input DMAs + layout
            # build run. N=256 keeps each one cheap. Targets the LAST bank
            # to be opened for real accumulation (b1 cp3, ~77us away).
            scratch = wpool.tile([128, 512], f16)
            nc.vector.memset(scratch[:], 0.0)
            for wi in range(NWARM):
                nc.tensor.matmul(
                    ps_t[BPC - 1][3][0:64, 0:256],
                    lhsT=scratch[:, 0:64],
                    rhs=scratch[:, 0:256],
                    start=(wi == 0),
                    stop=(wi == NWARM - 1),
                    skip_group_check=True,
                )

            # weights on the scalar DMA queue (sync queue carries images);
            # first 12 tiles separately so tap 0 unblocks early
            wsh_sb = wpool.tile([128, NT * 64], f16)
            nc.scalar.dma_start(out=wsh_sb[:, 0 : 12 * 64], in_=wsh[:, 0 : 12 * 64])
            nc.scalar.dma_start(out=wsh_sb[:, 12 * 64 :], in_=wsh[:, 12 * 64 :])

            xp_t, xq_t, wm_t = [], [], []
            for b in range(BPC):
                wm = wmpool.tile([128, 6 * 64], f16, name=f"wm{b}")
                nc.scalar.dma_start(out=wm[:], in_=wmod[b, :, :])
                # Contiguous DMA (8KB runs/partition, fast) of the image
                # into BOTH partition halves of a staging tile; the strided
                # padded layout is built on-chip by DVE (direct strided DMA
                # measured ~10x slower).
                xst = xstpool.tile([128, H, W], f16, name=f"xst{b}")
                nc.sync.dma_start(out=xst[0:64, :, :], in_=xs[b, :, :, :])
                nc.sync.dma_start(out=xst[64:128, :, :], in_=xs[b, :, :, :])

                xp = xppool.tile([128, HP, WP], f16, name=f"xp{b}")
                xq = xqpool.tile([128, HP, WP], f16, name=f"xq{b}")
                # border-only memsets, disjoint from the copied interiors;
                # they depend on nothing so they fill the DMA wait
                nc.vector.memset(xp[:, 0:PAD, :], 0.0)
                nc.vector.memset(xp[:, PAD + H :, :], 0.0)
                nc.vector.memset(xp[0:64, PAD : PAD + H, 0:PAD], 0.0)
                nc.vector.memset(xp[0:64, PAD : PAD + H, PAD + W :], 0.0)
                nc.vector.memset(xp[64:128, PAD : PAD + H, 0 : PAD - 1], 0.0)
                nc.vector.memset(xp[64:128, PAD : PAD + H, PAD - 1 + W :], 0.0)
                # xq borders (gpsimd: far off the critical path)
                nc.gpsimd.memset(xq[0:64, 0:PAD, :], 0.0)
                nc.gpsimd.memset(xq[0:64, PAD + H :, :], 0.0)
                nc.gpsimd.memset(xq[64:128, 0 : PAD - 1, :], 0.0)
                nc.gpsimd.memset(xq[64:128, PAD - 1 + H :, :], 0.0)
                nc.gpsimd.memset(xq[0:64, PAD : PAD + H, 0:PAD], 0.0)
                nc.gpsimd.memset(xq[0:64, PAD : PAD + H, PAD + W :], 0.0)
                nc.gpsimd.memset(xq[64:128, PAD - 1 : PAD - 1 + H, 0:PAD], 0.0)
                nc.gpsimd.memset(xq[64:128, PAD - 1 : PAD - 1 + H, PAD + W :], 0.0)
                # interiors on vector (fast): xp parts 0-63 = padded image,
                # 64-127 = shifted left 1 col; xq 64-127 = shifted up 1 row
                nc.vector.tensor_copy(
                    xp[0:64, PAD : PAD + H, PAD : PAD + W], xst[0:64, :, :]
                )
                nc.vector.tensor_copy(
                    xp[64:128, PAD : PAD + H, PAD - 1 : PAD - 1 + W],
                    xst[64:128, :, :],
                )
                nc.vector.tensor_copy(
                    xq[0:64, PAD : PAD + H, PAD : PAD + W], xst[0:64, :, :]
                )
                nc.vector.tensor_copy(
                    xq[64:128, PAD - 1 : PAD - 1 + H, PAD : PAD + W],
                    xst[64:128, :, :],
                )
                xp_t.append(xp)
                xq_t.append(xq)
                wm_t.append(wm)

            for b in range(BPC):
                for t in range(NT):
                    s = _MOD_SLOT.get(t)
                    w_ap = (
                        wm_t[b][:, s * 64 : (s + 1) * 64]
                        if s is not None
                        else wsh_sb[:, t * 64 : (t + 1) * 64]
                    )
                    i, j = _tile_tap(t)
                    xt = xq_t[b] if 78 <= t < 84 else xp_t[b]
                    for cp in range(4):
                        for half in (0, 1):
                            r0 = i + 8 * (2 * cp + half)
                            nc.tensor.matmul(
                                ps_t[b][cp][64 * half : 64 * (half + 1), :],
                                lhsT=w_ap,
                                rhs=xt[:, r0 : r0 + 8, j : j + 64],
                                start=(t == 0),
                                stop=(t == NT - 1),
                                tile_position=(0, 64 * half),
                                skip_group_check=True,
                            )
                for cp in range(4):
                    ot = opool.tile([128, CHUNK], f32, name=f"ot{b}{cp}")
                    nc.vector.tensor_copy(ot[:], ps_t[b][cp][:])
                    nc.sync.dma_start(
                        out=y[b, :, (2 * cp) * CHUNK : (2 * cp + 1) * CHUNK],
                        in_=ot[0:64, :],
                    )
                    nc.sync.dma_start(
                        out=y[b, :, (2 * cp + 1) * CHUNK : (2 * cp + 2) * CHUNK],
                        in_=ot[64:128, :],
                    )
    nc.compile()
    return nc


def _get_nc():
    global _CACHED_NC
    if _CACHED_NC is None:
        _CACHED_NC = _build_nc()
    return _CACHED_NC


def _host_dyn(x, w1, b1, w2, b2):
    """dwc_proj MLP on host, float64: dyn [B, 64, 9]."""
    pooled = x[:, :PDIM].mean(axis=(2, 3), dtype=np.float64)      # [B, 64]
    z = pooled @ w1.T.astype(np.float64) + b1.astype(np.float64)  # [B, 32]
    h = 0.5 * z * (1.0 + _ERF(z / math.sqrt(2.0)))                # exact gelu
    dyn = h @ w2.T.astype(np.float64) + b2.astype(np.float64)     # [B, 576]
    return dyn.reshape(B, PDIM, SK * SK)


def _host_weights(lk_filter, dyn):
    """Build shared tap-pair weight tiles + per-batch modified central tiles.

    Weight tile t [128, 64]: rows 0-63 = lk[o, c, iA, jA].T (tap A), rows
    64-127 = tap B, zeros for the single. lhsT layout [K=c, M=o].
    """
    lkT = lk_filter.transpose(1, 0, 2, 3).astype(np.float32)  # [c, o, i, j]
    Wt = np.zeros((NT, 128, 64), np.float32)
    for t in range(NT):
        i, jA = _tile_tap(t)
        Wt[t, 0:64, :] = lkT[:, :, i, jA]
        if t < 78:
            Wt[t, 64:128, :] = lkT[:, :, i, jA + 1]
        elif t < 84:
            Wt[t, 64:128, :] = lkT[:, :, i + 1, jA]

    ar = np.arange(64)
    Wmod = np.zeros((B, 6, 128, 64), np.float32)
    for ii, i in enumerate((5, 6, 7)):
        t2, t3 = i * 6 + 2, i * 6 + 3
        u = i - 5
        for b in range(B):
            m2 = Wt[t2].copy()
            m3 = Wt[t3].copy()
            m2[64 + ar, ar] += dyn[b, :, u * 3 + 0].astype(np.float32)  # tap (i,5)
            m3[ar, ar] += dyn[b, :, u * 3 + 1].astype(np.float32)       # tap (i,6)
            m3[64 + ar, ar] += dyn[b, :, u * 3 + 2].astype(np.float32)  # tap (i,7)
            Wmod[b, ii] = m2
            Wmod[b, 3 + ii] = m3

    wsh_np = np.ascontiguousarray(
        Wt.transpose(1, 0, 2).reshape(128, NT * 64)
    ).astype(np.float16)
    wmod_np = np.ascontiguousarray(
        Wmod.transpose(0, 2, 1, 3).reshape(B, 128, 6 * 64)
    ).astype(np.float16)
    return wsh_np, wmod_np


def kernel(x, lk_filter, w1, b1, w2, b2):
    from concourse.bass_utils import run_bass_kernel_spmd

    x = np.asarray(x, dtype=np.float32)
    dyn = _host_dyn(x, np.asarray(w1), np.asarray(b1), np.asarray(w2), np.asarray(b2))
    wsh_np, wmod_np = _host_weights(np.asarray(lk_filter, dtype=np.float32), dyn)

    x1_f16 = x[:, :PDIM].astype(np.float16)  # [16, 64, 64, 64]

    nc = _get_nc()
    in_maps = []
    for k in range(NCORES):
        b0 = k * BPC
        in_maps.append(
            {
                "xs": np.ascontiguousarray(x1_f16[b0 : b0 + BPC]),
                "wsh": wsh_np,
                "wmod": np.ascontiguousarray(wmod_np[b0 : b0 + BPC]),
            }
        )
    res = run_bass_kernel_spmd(nc, in_maps, core_ids=list(range(NCORES)))

    out = np.empty((B, C, H, W), np.float32)
    for k in range(NCORES):
        b0 = k * BPC
        out[b0 : b0 + BPC, :PDIM] = res.results[k]["y"].reshape(BPC, PDIM, H, W)
    out[:, PDIM:] = x[:, PDIM:]
    return out
